# Initial kernel scaffold
#
"""Trainium2 Bass kernel for nn_Attention_57432302682539.

Reference computation (B=32, S=4096, D=256, H=256):
    inp = x @ W_in.T + b_in                                  # [B, H]
    branch_i: ctx = einsum('bsd,hd->bhs', context, Wc_i) + bc_i
              att_i = einsum('h,bhs->bs', V_i, tanh(inp[:,:,None] + ctx))
    att = concat(att_0..3, axis=1)                           # [B, 4S]
    att = 10*tanh(att)  (mask is all ones -> where() is identity)
    out = softmax(att, axis=0)                               # over batch

Sharding: S is split 8 ways (512 positions per core); every core holds all
32 batches, so the dim-0 (batch) softmax is entirely core-local and no
collective is needed.

Per-core pipeline:
  - main matmuls: WcT.T @ ctxT -> PSUM [128h, 512s], K=256 via 2 accum steps
  - ACT tanh with per-partition bias (inp[b]+bc) -> SBUF fp16 tile
  - V-dot: matmul with a zero-padded sliding-window V buffer whose single
    nonzero column routes each (b,br) scalar row into partition br*32+b of a
    single PSUM accumulator [128, 512]
  - exp(10*tanh(att)) on ACT, partition-group sums via a 0/1 select matmul,
    DVE reciprocal, DMA partition-broadcast, DVE multiply, DMA out.
"""

import os
import numpy as np

B, S, D, H = 32, 4096, 256, 256
NCORES = 8
SC = S // NCORES          # 512 s-positions per core
NBR = 4                   # branches
HT = 2                    # h tiles of 128
DTILES = 2                # d tiles of 128
P = 128

_CACHE = {}


def _build_nc(dt_name: str):
    """Build the Bass module. dt_name in ('float16', 'bfloat16')."""
    import concourse.bass as bass
    import concourse.mybir as mybir
    import concourse.tile as tile

    DT = getattr(mybir.dt, dt_name)
    F32 = mybir.dt.float32
    AF = mybir.ActivationFunctionType

    nc = bass.Bass(trn_type="TRN2")

    # Per-core external inputs (host-preprocessed).
    ctxT = nc.dram_tensor("ctxT", [B, DTILES, P, SC], DT, kind="ExternalInput")
    wcT = nc.dram_tensor("wcT", [P, DTILES, NBR, HT, P], DT, kind="ExternalInput")
    vbuf = nc.dram_tensor("vbuf", [P, NBR * HT, 256], DT, kind="ExternalInput")
    winT = nc.dram_tensor("winT", [P, DTILES, HT, P], DT, kind="ExternalInput")
    xT = nc.dram_tensor("xT", [P, DTILES, B], DT, kind="ExternalInput")
    bcomb = nc.dram_tensor("bcomb", [P, NBR, HT], F32, kind="ExternalInput")
    sel = nc.dram_tensor("sel", [P, NBR], F32, kind="ExternalInput")
    out = nc.dram_tensor("out", [B, NBR, SC], F32, kind="ExternalOutput")

    from contextlib import ExitStack

    with tile.TileContext(nc) as tc, ExitStack() as ctx:
        const = ctx.enter_context(tc.tile_pool(name="const", bufs=1))
        ctxp = ctx.enter_context(tc.tile_pool(name="ctxp", bufs=6))
        tanhp = ctx.enter_context(tc.tile_pool(name="tanhp", bufs=8))
        finalp = ctx.enter_context(tc.tile_pool(name="finalp", bufs=1))
        ps_main = ctx.enter_context(tc.tile_pool(name="ps_main", bufs=4, space="PSUM"))
        ps_att = ctx.enter_context(tc.tile_pool(name="ps_att", bufs=1, space="PSUM"))
        ps_small = ctx.enter_context(tc.tile_pool(name="ps_small", bufs=2, space="PSUM"))
        ps_den = ctx.enter_context(tc.tile_pool(name="ps_den", bufs=1, space="PSUM"))

        # ---- constants into SBUF ----
        wcT_sb = const.tile([P, DTILES, NBR, HT, P], DT)
        nc.sync.dma_start(out=wcT_sb, in_=wcT[:])
        vbuf_sb = const.tile([P, NBR * HT, 256], DT)
        nc.sync.dma_start(out=vbuf_sb, in_=vbuf[:])
        winT_sb = const.tile([P, DTILES, HT, P], DT)
        nc.sync.dma_start(out=winT_sb, in_=winT[:])
        xT_sb = const.tile([P, DTILES, B], DT)
        nc.sync.dma_start(out=xT_sb, in_=xT[:])
        bcomb_sb = const.tile([P, NBR, HT], F32)
        nc.sync.dma_start(out=bcomb_sb, in_=bcomb[:])
        sel_sb = const.tile([P, NBR], F32)
        nc.sync.dma_start(out=sel_sb, in_=sel[:])

        # ---- inp = x @ W_in.T (+ b_in + bc), laid h-on-partitions ----
        # bias_all[:, br, ht, b] = sum_d W_in[h,d] x[b,d] + b_in[h] + bc_br[h]
        bias_all = const.tile([P, NBR, HT, B], F32)
        for ht in range(HT):
            ps_inp = ps_small.tile([P, B], F32, name="ps_inp")
            for dti in range(DTILES):
                nc.tensor.matmul(
                    ps_inp[:],
                    lhsT=winT_sb[:, dti, ht],
                    rhs=xT_sb[:, dti],
                    start=(dti == 0),
                    stop=(dti == DTILES - 1),
                )
            for br in range(NBR):
                nc.vector.tensor_scalar_add(
                    bias_all[:, br, ht], ps_inp[:], bcomb_sb[:, br, ht : ht + 1]
                )

        # ---- main pipeline over (b, br, ht) ----
        att_ps = ps_att.tile([P, SC], F32)

        tiles = [(b, br, ht) for b in range(B) for br in range(NBR) for ht in range(HT)]
        ntiles = len(tiles)
        LAG = 3  # software pipelining distance for the V-dot matmuls
        pending = []  # (tanh_tile, b, br, ht)
        vdot_idx = 0

        def emit_vdot(tt, b, br, ht, idx):
            j = br * 32 + b
            k = br * HT + ht
            nc.tensor.matmul(
                att_ps[:],
                lhsT=vbuf_sb[:, k, 127 - j : 255 - j],
                rhs=tt[:],
                start=(idx == 0),
                stop=(idx == ntiles - 1),
            )

        ctx_tiles = {}
        for i, (b, br, ht) in enumerate(tiles):
            if (br, ht) == (0, 0):
                # new batch: load both d-tiles of transposed context
                t0 = ctxp.tile([P, SC], DT, tag="ctx", name="ctx0")
                t1 = ctxp.tile([P, SC], DT, tag="ctx", name="ctx1")
                nc.sync.dma_start(out=t0, in_=ctxT[b, 0])
                nc.sync.dma_start(out=t1, in_=ctxT[b, 1])
                ctx_tiles = {0: t0, 1: t1}
            ps = ps_main.tile([P, SC], F32, name="ps_ctx")
            for dti in range(DTILES):
                nc.tensor.matmul(
                    ps[:],
                    lhsT=wcT_sb[:, dti, br, ht],
                    rhs=ctx_tiles[dti][:],
                    start=(dti == 0),
                    stop=(dti == DTILES - 1),
                )
            tt = tanhp.tile([P, SC], DT, tag="tanh", name="tt")
            nc.scalar.activation(
                tt[:], ps[:], AF.Tanh, bias=bias_all[:, br, ht, b : b + 1]
            )
            pending.append((tt, b, br, ht))
            if i >= LAG:
                ptt, pb, pbr, pht = pending[vdot_idx]
                emit_vdot(ptt, pb, pbr, pht, vdot_idx)
                vdot_idx += 1
        while vdot_idx < ntiles:
            ptt, pb, pbr, pht = pending[vdot_idx]
            emit_vdot(ptt, pb, pbr, pht, vdot_idx)
            vdot_idx += 1

        # ---- softmax over batch (local: all 32 batches are on this core) ----
        # att rows are laid p = br*32 + b
        th = finalp.tile([P, SC], F32)
        nc.scalar.activation(th[:], att_ps[:], AF.Tanh)
        ex = finalp.tile([P, SC], F32)
        nc.scalar.activation(ex[:], th[:], AF.Exp, scale=10.0)

        den_ps = ps_den.tile([NBR, SC], F32)
        nc.tensor.matmul(den_ps[:], lhsT=sel_sb[:], rhs=ex[:], start=True, stop=True)

        inv = finalp.tile([NBR, SC], F32)
        nc.vector.reciprocal(inv[:], den_ps[:])

        # broadcast inv [4, SC] -> [128, SC]: partition p reads inv[p // 32]
        invrep = finalp.tile([P, SC], F32)
        inv_bcast = bass.AP(
            tensor=inv.tensor,
            offset=inv.offset,
            ap=[inv.ap[0], [0, 32], inv.ap[1]],
        )
        nc.sync.dma_start(out=invrep, in_=inv_bcast)

        outv = finalp.tile([P, SC], F32)
        nc.vector.tensor_mul(outv[:], ex[:], invrep[:])

        for br in range(NBR):
            nc.sync.dma_start(out=out[:, br, :], in_=outv[br * 32 : (br + 1) * 32, :])

    return nc


def _host_prep(inputs, np_dt):
    """Build the per-core input maps from the full problem inputs."""
    x = np.asarray(inputs["x"], np.float32)
    context = np.ascontiguousarray(np.asarray(inputs["context"], np.float32))
    W_in = np.asarray(inputs["W_in"], np.float32)
    b_in = np.asarray(inputs["b_in"], np.float32)
    Wc = np.stack(
        [np.asarray(inputs[f"Wc{i}"], np.float32) for i in range(NBR)]
    )  # [br, h, d]
    bc = np.stack([np.asarray(inputs[f"bc{i}"], np.float32) for i in range(NBR)])
    V = np.stack([np.asarray(inputs[f"V{i}"], np.float32) for i in range(NBR)])

    # wcT[p, dt, br, ht, j] = Wc[br, ht*128+j, dt*128+p]
    wcT = np.ascontiguousarray(
        Wc.reshape(NBR, HT, P, DTILES, P).transpose(4, 3, 0, 1, 2)
    ).astype(np_dt)

    # vbuf[p, br*2+ht, 127] = V[br, ht*128+p]; zero elsewhere
    vbuf = np.zeros((P, NBR * HT, 256), np.float32)
    for br in range(NBR):
        for ht in range(HT):
            vbuf[:, br * HT + ht, 127] = V[br, ht * P : (ht + 1) * P]
    vbuf = vbuf.astype(np_dt)

    # winT[p, dt, ht, j] = W_in[ht*128+j, dt*128+p]
    winT = np.ascontiguousarray(
        W_in.reshape(HT, P, DTILES, P).transpose(3, 2, 0, 1)
    ).astype(np_dt)

    # xT[p, dt, b] = x[b, dt*128+p]
    xT = np.ascontiguousarray(x.reshape(B, DTILES, P).transpose(2, 1, 0)).astype(np_dt)

    # bcomb[p, br, ht] = b_in[ht*128+p] + bc[br, ht*128+p]
    bsum = b_in[None, :] + bc  # [br, H]
    bcomb = np.ascontiguousarray(
        bsum.reshape(NBR, HT, P).transpose(2, 0, 1)
    ).astype(np.float32)

    # sel[p, m] = 1 if p//32 == m
    sel = np.zeros((P, NBR), np.float32)
    for m in range(NBR):
        sel[m * 32 : (m + 1) * 32, m] = 1.0

    shared = dict(wcT=wcT, vbuf=vbuf, winT=winT, xT=xT, bcomb=bcomb, sel=sel)

    in_maps = []
    for k in range(NCORES):
        sl = context[:, k * SC : (k + 1) * SC, :]  # [B, SC, D]
        ctxT = np.ascontiguousarray(sl.transpose(0, 2, 1)).astype(np_dt)  # [B, D, SC]
        m = dict(shared)
        m["ctxT"] = ctxT.reshape(B, DTILES, P, SC)
        in_maps.append(m)
    return in_maps


def kernel(**inputs) -> np.ndarray:
    dt_name = os.environ.get("KERNEL_DT", "float16")
    np_dt = {"float16": np.float16, "bfloat16": None}[dt_name]
    if np_dt is None:
        import ml_dtypes

        np_dt = ml_dtypes.bfloat16

    from concourse import bass_utils

    if dt_name not in _CACHE:
        _CACHE[dt_name] = _build_nc(dt_name)
    nc = _CACHE[dt_name]

    in_maps = _host_prep(inputs, np_dt)
    res = bass_utils.run_bass_kernel_spmd(nc, in_maps, core_ids=list(range(NCORES)))

    full = np.empty((B, NBR, NCORES, SC), np.float32)
    for k in range(NCORES):
        full[:, :, k, :] = res.results[k]["out"]
    return full.reshape(B, NBR * S).astype(np.float32)


if __name__ == "__main__":
    # smoke test with random inputs
    rng = np.random.default_rng(0)
    inputs = dict(
        x=rng.standard_normal((B, H), dtype=np.float32),
        context=rng.standard_normal((B, S, D), dtype=np.float32),
        mask=np.ones((B, S), bool),
        W_in=rng.uniform(-1 / 16, 1 / 16, (H, H)).astype(np.float32),
        b_in=rng.uniform(-1 / 16, 1 / 16, (H,)).astype(np.float32),
    )
    for i in range(4):
        inputs[f"Wc{i}"] = rng.uniform(-1 / 16, 1 / 16, (H, D)).astype(np.float32)
        inputs[f"bc{i}"] = rng.uniform(-1 / 16, 1 / 16, (H,)).astype(np.float32)
        inputs[f"V{i}"] = rng.uniform(-1, 1, (H,)).astype(np.float32)
    out = kernel(**inputs)
    print("out", out.shape, out.dtype, out.sum())


# revision 7
# speedup vs baseline: 69.2154x; 69.2154x over previous
"""Trainium2 Bass kernel for nn_Attention_57432302682539.

Reference computation (B=32, S=4096, D=256, H=256):
    inp = x @ W_in.T + b_in                                  # [B, H]
    branch_i: ctx = einsum('bsd,hd->bhs', context, Wc_i) + bc_i
              att_i = einsum('h,bhs->bs', V_i, tanh(inp[:,:,None] + ctx))
    att = concat(att_0..3, axis=1)                           # [B, 4S]
    att = 10*tanh(att)  (mask is all ones -> where() is identity)
    out = softmax(att, axis=0)                               # over batch

Sharding: S is split 8 ways (512 positions per core); every core holds all
32 batches, so the dim-0 (batch) softmax is entirely core-local and no
collective is needed.

Per-core pipeline:
  - main matmuls: WcT.T @ ctxT -> PSUM [128h, 512s], K=256 via 2 accum steps
  - ACT tanh with per-partition bias (inp[b]+bc) -> SBUF fp16 tile
  - V-dot: matmul with a zero-padded sliding-window V buffer whose single
    nonzero column routes each (b,br) scalar row into partition br*32+b of a
    single PSUM accumulator [128, 512]
  - exp(10*tanh(att)) on ACT, partition-group sums via a 0/1 select matmul,
    DVE reciprocal, DMA partition-broadcast, DVE multiply, DMA out.
"""

import os
import numpy as np

B, S, D, H = 32, 4096, 256, 256
NCORES = 8
SC = S // NCORES          # 512 s-positions per core
NBR = 4                   # branches
HT = 2                    # h tiles of 128
DTILES = 2                # d tiles of 128
P = 128

_CACHE = {}


def _build_nc(dt_name: str, repeat: int = 1):
    """Build the Bass module. dt_name in ('float16', 'bfloat16').

    repeat>1 unrolls the whole computation N times inside the NEFF (for
    on-device timing via wall-clock differencing); the result is unchanged.
    """
    import concourse.bass as bass
    import concourse.mybir as mybir
    import concourse.tile as tile
    from concourse import bacc

    DT = getattr(mybir.dt, dt_name)
    F32 = mybir.dt.float32
    AF = mybir.ActivationFunctionType

    nc = bacc.Bacc(trn_type="TRN2")

    # Per-core external inputs (host-preprocessed).
    ctxT = nc.dram_tensor("ctxT", [B, DTILES, P, SC], DT, kind="ExternalInput")
    wcT = nc.dram_tensor("wcT", [P, DTILES, NBR, HT, P], DT, kind="ExternalInput")
    vbuf = nc.dram_tensor("vbuf", [P, NBR * HT, 256], DT, kind="ExternalInput")
    winT = nc.dram_tensor("winT", [P, DTILES, HT, P], DT, kind="ExternalInput")
    xT = nc.dram_tensor("xT", [P, DTILES, B], DT, kind="ExternalInput")
    bcomb = nc.dram_tensor("bcomb", [P, NBR, HT], F32, kind="ExternalInput")
    sel = nc.dram_tensor("sel", [P, NBR], F32, kind="ExternalInput")
    out = nc.dram_tensor("out", [B, NBR, SC], F32, kind="ExternalOutput")

    from contextlib import ExitStack

    with tile.TileContext(nc) as tc, ExitStack() as ctx:
        const = ctx.enter_context(tc.tile_pool(name="const", bufs=1))
        ctxp = ctx.enter_context(tc.tile_pool(name="ctxp", bufs=6))
        tanhp = ctx.enter_context(tc.tile_pool(name="tanhp", bufs=8))
        finalp = ctx.enter_context(tc.tile_pool(name="finalp", bufs=1))
        ps_main = ctx.enter_context(tc.tile_pool(name="ps_main", bufs=4, space="PSUM"))
        ps_att = ctx.enter_context(tc.tile_pool(name="ps_att", bufs=1, space="PSUM"))
        ps_small = ctx.enter_context(tc.tile_pool(name="ps_small", bufs=2, space="PSUM"))
        ps_den = ctx.enter_context(tc.tile_pool(name="ps_den", bufs=1, space="PSUM"))

        # ---- constants into SBUF ----
        wcT_sb = const.tile([P, DTILES, NBR, HT, P], DT)
        nc.gpsimd.dma_start(out=wcT_sb, in_=wcT[:])
        vbuf_sb = const.tile([P, NBR * HT, 256], DT)
        nc.gpsimd.dma_start(out=vbuf_sb, in_=vbuf[:])
        winT_sb = const.tile([P, DTILES, HT, P], DT)
        nc.gpsimd.dma_start(out=winT_sb, in_=winT[:])
        xT_sb = const.tile([P, DTILES, B], DT)
        nc.gpsimd.dma_start(out=xT_sb, in_=xT[:])
        bcomb_sb = const.tile([P, NBR, HT], F32)
        nc.gpsimd.dma_start(out=bcomb_sb, in_=bcomb[:])
        sel_sb = const.tile([P, NBR], F32)
        nc.gpsimd.dma_start(out=sel_sb, in_=sel[:])

        # ---- inp = x @ W_in.T (+ b_in + bc), laid h-on-partitions ----
        # bias_all[:, br, ht, b] = sum_d W_in[h,d] x[b,d] + b_in[h] + bc_br[h]
        bias_all = const.tile([P, NBR, HT, B], F32)
        for ht in range(HT):
            ps_inp = ps_small.tile([P, B], F32, name="ps_inp")
            for dti in range(DTILES):
                nc.tensor.matmul(
                    ps_inp[:],
                    lhsT=winT_sb[:, dti, ht],
                    rhs=xT_sb[:, dti],
                    start=(dti == 0),
                    stop=(dti == DTILES - 1),
                )
            for br in range(NBR):
                nc.scalar.add(
                    bias_all[:, br, ht], ps_inp[:], bcomb_sb[:, br, ht : ht + 1]
                )

        # ---- main pipeline over (b, br, ht) ----
        tiles = [(b, br, ht) for b in range(B) for br in range(NBR) for ht in range(HT)]
        ntiles = len(tiles)
        LAG = 3  # software pipelining distance for the V-dot matmuls

        for _rep in range(repeat):
            att_ps = ps_att.tile([P, SC], F32, tag="att", name="att_ps")
            pending = []  # (tanh_tile, b, br, ht)
            vdot_idx = 0

            def emit_vdot(tt, b, br, ht, idx):
                j = br * 32 + b
                k = br * HT + ht
                nc.tensor.matmul(
                    att_ps[:],
                    lhsT=vbuf_sb[:, k, 127 - j : 255 - j],
                    rhs=tt[:],
                    start=(idx == 0),
                    stop=(idx == ntiles - 1),
                )

            ctx_tiles = {}
            for i, (b, br, ht) in enumerate(tiles):
                if (br, ht) == (0, 0):
                    # new batch: load both d-tiles of transposed context
                    t0 = ctxp.tile([P, SC], DT, tag="ctx", name="ctx0")
                    t1 = ctxp.tile([P, SC], DT, tag="ctx", name="ctx1")
                    nc.sync.dma_start(out=t0, in_=ctxT[b, 0])
                    nc.sync.dma_start(out=t1, in_=ctxT[b, 1])
                    ctx_tiles = {0: t0, 1: t1}
                ps = ps_main.tile([P, SC], F32, name="ps_ctx")
                for dti in range(DTILES):
                    nc.tensor.matmul(
                        ps[:],
                        lhsT=wcT_sb[:, dti, br, ht],
                        rhs=ctx_tiles[dti][:],
                        start=(dti == 0),
                        stop=(dti == DTILES - 1),
                    )
                tt = tanhp.tile([P, SC], DT, tag="tanh", name="tt")
                nc.scalar.activation(
                    tt[:], ps[:], AF.Tanh, bias=bias_all[:, br, ht, b : b + 1]
                )
                pending.append((tt, b, br, ht))
                if i >= LAG:
                    ptt, pb, pbr, pht = pending[vdot_idx]
                    emit_vdot(ptt, pb, pbr, pht, vdot_idx)
                    vdot_idx += 1
            while vdot_idx < ntiles:
                ptt, pb, pbr, pht = pending[vdot_idx]
                emit_vdot(ptt, pb, pbr, pht, vdot_idx)
                vdot_idx += 1

            # ---- softmax over batch (local: all 32 batches on this core) ----
            # att rows are laid p = br*32 + b
            th = finalp.tile([P, SC], F32, tag="th", name="th")
            nc.scalar.activation(th[:], att_ps[:], AF.Tanh)
            ex = finalp.tile([P, SC], F32, tag="ex", name="ex")
            nc.scalar.activation(ex[:], th[:], AF.Exp, scale=10.0)

            den_ps = ps_den.tile([NBR, SC], F32, tag="den", name="den_ps")
            nc.tensor.matmul(
                den_ps[:], lhsT=sel_sb[:], rhs=ex[:], start=True, stop=True
            )

            inv = finalp.tile([NBR, SC], F32, tag="inv", name="inv")
            nc.vector.reciprocal(inv[:], den_ps[:])

            # broadcast inv [4, SC] -> [128, SC]: partition p reads inv[p//32]
            invrep = finalp.tile([P, SC], F32, tag="invrep", name="invrep")
            inv_bcast = bass.AP(
                tensor=inv.tensor,
                offset=inv.offset,
                ap=[inv.ap[0], [0, 32], inv.ap[1]],
            )
            nc.sync.dma_start(out=invrep, in_=inv_bcast)

            outv = finalp.tile([P, SC], F32, tag="outv", name="outv")
            nc.vector.tensor_mul(outv[:], ex[:], invrep[:])

            for br in range(NBR):
                nc.sync.dma_start(
                    out=out[:, br, :], in_=outv[br * 32 : (br + 1) * 32, :]
                )

    nc.compile()
    return nc


def _host_prep(inputs, np_dt):
    """Build the per-core input maps from the full problem inputs."""
    x = np.asarray(inputs["x"], np.float32)
    context = np.ascontiguousarray(np.asarray(inputs["context"], np.float32))
    W_in = np.asarray(inputs["W_in"], np.float32)
    b_in = np.asarray(inputs["b_in"], np.float32)
    Wc = np.stack(
        [np.asarray(inputs[f"Wc{i}"], np.float32) for i in range(NBR)]
    )  # [br, h, d]
    bc = np.stack([np.asarray(inputs[f"bc{i}"], np.float32) for i in range(NBR)])
    V = np.stack([np.asarray(inputs[f"V{i}"], np.float32) for i in range(NBR)])

    # wcT[p, dt, br, ht, j] = Wc[br, ht*128+j, dt*128+p]
    wcT = np.ascontiguousarray(
        Wc.reshape(NBR, HT, P, DTILES, P).transpose(4, 3, 0, 1, 2)
    ).astype(np_dt)

    # vbuf[p, br*2+ht, 127] = V[br, ht*128+p]; zero elsewhere
    vbuf = np.zeros((P, NBR * HT, 256), np.float32)
    for br in range(NBR):
        for ht in range(HT):
            vbuf[:, br * HT + ht, 127] = V[br, ht * P : (ht + 1) * P]
    vbuf = vbuf.astype(np_dt)

    # winT[p, dt, ht, j] = W_in[ht*128+j, dt*128+p]
    winT = np.ascontiguousarray(
        W_in.reshape(HT, P, DTILES, P).transpose(3, 2, 0, 1)
    ).astype(np_dt)

    # xT[p, dt, b] = x[b, dt*128+p]
    xT = np.ascontiguousarray(x.reshape(B, DTILES, P).transpose(2, 1, 0)).astype(np_dt)

    # bcomb[p, br, ht] = b_in[ht*128+p] + bc[br, ht*128+p]
    bsum = b_in[None, :] + bc  # [br, H]
    bcomb = np.ascontiguousarray(
        bsum.reshape(NBR, HT, P).transpose(2, 0, 1)
    ).astype(np.float32)

    # sel[p, m] = 1 if p//32 == m
    sel = np.zeros((P, NBR), np.float32)
    for m in range(NBR):
        sel[m * 32 : (m + 1) * 32, m] = 1.0

    shared = dict(wcT=wcT, vbuf=vbuf, winT=winT, xT=xT, bcomb=bcomb, sel=sel)

    in_maps = []
    for k in range(NCORES):
        sl = context[:, k * SC : (k + 1) * SC, :]  # [B, SC, D]
        ctxT = np.ascontiguousarray(sl.transpose(0, 2, 1)).astype(np_dt)  # [B, D, SC]
        m = dict(shared)
        m["ctxT"] = ctxT.reshape(B, DTILES, P, SC)
        in_maps.append(m)
    return in_maps


def kernel(**inputs) -> np.ndarray:
    dt_name = os.environ.get("KERNEL_DT", "float16")
    np_dt = {"float16": np.float16, "bfloat16": None}[dt_name]
    if np_dt is None:
        import ml_dtypes

        np_dt = ml_dtypes.bfloat16

    from concourse import bass_utils

    if dt_name not in _CACHE:
        _CACHE[dt_name] = _build_nc(dt_name)
    nc = _CACHE[dt_name]

    in_maps = _host_prep(inputs, np_dt)
    res = bass_utils.run_bass_kernel_spmd(nc, in_maps, core_ids=list(range(NCORES)))

    full = np.empty((B, NBR, NCORES, SC), np.float32)
    for k in range(NCORES):
        full[:, :, k, :] = res.results[k]["out"]
    return full.reshape(B, NBR * S).astype(np.float32)


if __name__ == "__main__":
    # smoke test with random inputs
    rng = np.random.default_rng(0)
    inputs = dict(
        x=rng.standard_normal((B, H), dtype=np.float32),
        context=rng.standard_normal((B, S, D), dtype=np.float32),
        mask=np.ones((B, S), bool),
        W_in=rng.uniform(-1 / 16, 1 / 16, (H, H)).astype(np.float32),
        b_in=rng.uniform(-1 / 16, 1 / 16, (H,)).astype(np.float32),
    )
    for i in range(4):
        inputs[f"Wc{i}"] = rng.uniform(-1 / 16, 1 / 16, (H, D)).astype(np.float32)
        inputs[f"bc{i}"] = rng.uniform(-1 / 16, 1 / 16, (H,)).astype(np.float32)
        inputs[f"V{i}"] = rng.uniform(-1, 1, (H,)).astype(np.float32)
    out = kernel(**inputs)
    print("out", out.shape, out.dtype, out.sum())
